# revision 32
# baseline (speedup 1.0000x reference)
"""Trainium2 Bass kernel for nn_MixedRationalQuadraticCouplingTransform.

kernel(**inputs) takes FULL inputs (N=65536), returns (outputs [N,128] f32,
logabsdet [N] f32). Pure data parallel on 8 NeuronCores: batch sharded 8 ways,
weights replicated.

Per-core program (Tile framework):
  - 16 tiles of 512 samples (4 sub-chunks of 128).
  - net_in (cos/sin/ir/ctx) is precomputed on host; on-chip it is transposed
    to feature-major via PE transposes, then a 3-layer MLP runs with fp32r
    matmul operands (full-rate on the PE, ~1.6e-4 matmul precision).
    Layer 3 uses feature-major h2 as the stationary operand so the 992+pad
    spline params land sample-major in PSUM, with W3 columns host-permuted
    into [uw_c|uh_c|ud_c|pad|uw_r|uh_r|ud_r] blocks and b3 applied via an
    appended ones-row contraction.
  - Both splines (circular + regular) are evaluated by shared ops over a
    64-wide feature axis with per-feature constant tiles; bin search is
    mask-based (m_j = [x >= knot_j]) and gathers are mask-dot products
    evaluated with a pairwise adder tree. All activation functions used
    (Exp/Ln/Relu/Copy/Identity/Square) live in one ACT table so the table is
    loaded exactly once; softplus(x) = Ln(Exp(x) + 1).
"""
import numpy as np

import concourse.bacc as bacc
import concourse.tile as tile
from concourse import mybir
from concourse.bass_utils import run_bass_kernel_spmd

F32 = mybir.dt.float32
F16 = mybir.dt.float16
F32R = mybir.dt.float32r
AX = mybir.AxisListType
OP = mybir.AluOpType
AF = mybir.ActivationFunctionType

N_FULL = 65536
N_CORES = 8
N_CORE = N_FULL // N_CORES          # 8192
S_TILE = 512
C_SUB = S_TILE // 128               # 4
PI = float(np.pi)
SCALE = float(1.0 / np.sqrt(512.0))
MIN_BW = 1e-3
MIN_D = 1e-3
K = 5
TAIL = 5.0
DT_W = F32   # dtype of wide spline tensors

_TABLE_PATCHED = False


def _patch_single_act_table():
    """Force the act-table pass to use only natural_log_exp_and_others
    (covers Exp/Ln/Relu/Copy/Identity/Square) so the table loads once."""
    global _TABLE_PATCHED
    if _TABLE_PATCHED:
        return
    from concourse.hw_specs import get_activation_tables as _orig

    def single(arch):
        tabs = _orig(arch)
        keep = "natural_log_exp_and_others"
        return {k: (v if k == keep else set()) for k, v in tabs.items()}

    bacc.get_activation_tables = single
    _TABLE_PATCHED = True


def build_nc(n_core=N_CORE):
    _patch_single_act_table()
    n_tiles = n_core // S_TILE
    C = C_SUB
    CD = C * 64
    nc = bacc.Bacc("TRN2", target_bir_lowering=False, debug=False,
                   enable_asserts=True, num_devices=1)

    d_in = nc.dram_tensor("inputs", [n_core, 128], F32, kind="ExternalInput").ap()
    d_net = nc.dram_tensor("net32", [n_core, 160], F32R, kind="ExternalInput").ap()
    d_w1 = nc.dram_tensor("W1f", [160, 512], F32R, kind="ExternalInput").ap()
    d_w2 = nc.dram_tensor("W2f", [512, 512], F32R, kind="ExternalInput").ap()
    d_w3 = nc.dram_tensor("W3b", [513, 1024], F32R, kind="ExternalInput").ap()
    d_b1 = nc.dram_tensor("b1r", [128, 4], F32, kind="ExternalInput").ap()
    d_b2 = nc.dram_tensor("b2r", [128, 4], F32, kind="ExternalInput").ap()
    d_id = nc.dram_tensor("ident", [128, 128], F32R, kind="ExternalInput").ap()
    d_ones = nc.dram_tensor("onesr", [1, 128], F32R, kind="ExternalInput").ap()
    # per-feature constants over the merged 64-wide transform axis
    d_cL = nc.dram_tensor("cL", [128, 64], F32, kind="ExternalInput").ap()
    d_cW = nc.dram_tensor("cW", [128, 64], F32, kind="ExternalInput").ap()
    d_xn = nc.dram_tensor("xn", [n_core, 64], F32, kind="ExternalInput").ap()
    d_out = nc.dram_tensor("outputs", [n_core, 128], F32, kind="ExternalOutput").ap()
    d_lad = nc.dram_tensor("lad", [n_core], F32, kind="ExternalOutput").ap()

    with tile.TileContext(nc) as tc:
        with tc.tile_pool(name="const", bufs=1) as cst, \
             tc.tile_pool(name="mlp", bufs=2) as mlp, \
             tc.tile_pool(name="sm", bufs=1) as sm, \
             tc.tile_pool(name="lg", bufs=1) as lg, \
             tc.tile_pool(name="pt", bufs=1, space="PSUM") as ppt, \
             tc.tile_pool(name="pmlp", bufs=3, space="PSUM") as pmlp, \
             tc.tile_pool(name="pl3", bufs=1, space="PSUM") as pl3:

            vec, gps, act = nc.vector, nc.gpsimd, nc.scalar

            # ---- identity feature columns: straight DRAM->DRAM ----
            nc.sync.dma_start(d_out[:, 0:32], d_in[:, 0:32])
            nc.sync.dma_start(d_out[:, 64:96], d_in[:, 64:96])

            # ---- constants ----
            w1t = cst.tile([128, 512], F32R)
            nc.sync.dma_start(w1t[:], d_w1[0:128, :])
            w1b = cst.tile([32, 512], F32R)
            nc.sync.dma_start(w1b[:], d_w1[128:160, :])
            w2t = cst.tile([128, 4, 512], F32R)
            nc.sync.dma_start(w2t[:], d_w2.rearrange("(k p) h -> p k h", p=128))
            w3t = cst.tile([128, 4, 1024], F32R)
            nc.sync.dma_start(w3t[:], d_w3[0:512, :].rearrange("(k p) n -> p k n", p=128))
            w3l = cst.tile([1, 1024], F32R)
            nc.sync.dma_start(w3l[:], d_w3[512:513, :])
            b1t = cst.tile([128, 4], F32)
            nc.sync.dma_start(b1t[:], d_b1[:])
            b2t = cst.tile([128, 4], F32)
            nc.sync.dma_start(b2t[:], d_b2[:])
            idt = cst.tile([128, 128], F32R)
            nc.sync.dma_start(idt[:], d_id[:])
            cL = cst.tile([128, 64], F32)
            nc.sync.dma_start(cL[:], d_cL[:])
            cW = cst.tile([128, 64], F32)
            nc.sync.dma_start(cW[:], d_cW[:])
            ones = cst.tile([1, 128], F32R)
            nc.sync.dma_start(ones[:], d_ones[:])

            for ts_ in range(n_tiles // 2):
                CS = 2 * C          # 8 sub-chunks per spline supertile
                CDS = CS * 64
                R0 = ts_ * 2 * S_TILE
                # spline-wide tiles for the supertile (e4 doubles as wh in place)
                e4 = lg.tile([128, CS * 2, 64, 5], F32, tag="e4", bufs=1)
                sp = lg.tile([128, CS, 64, 6], F32, tag="sp", bufs=1)

                for sub in range(2):
                    r0 = R0 + sub * S_TILE
                    nin = sm.tile([128, C, 160], F32R, tag="nin", bufs=1)
                    nc.sync.dma_start(
                        nin[:],
                        d_net[r0:r0 + S_TILE, :].rearrange("(c p) f -> p c f", p=128))

                    # transpose net_in to feature-major
                    ntA = sm.tile([128, C, 128], F32R, tag="ntA", bufs=1)
                    ntB = sm.tile([32, C, 128], F32R, tag="ntB", bufs=1)
                    for c in range(C):
                        pt = ppt.tile([128, 256], F32R, tag="pt")
                        nc.tensor.transpose(pt[:, 0:128], nin[:, c, 0:128], idt[:])
                        nc.tensor.transpose(pt[0:32, 128:256], nin[:, c, 128:160],
                                            idt[:])
                        act.copy(ntA[:, c, :], pt[:, 0:128])
                        act.copy(ntB[:, c, :], pt[0:32, 128:256])
                    ntA_f = ntA[:].rearrange("p c s -> p (c s)")
                    ntB_f = ntB[:].rearrange("p c s -> p (c s)")

                    # layer 1
                    h1t = mlp.tile([128, 4, 512], F32R, tag="h1", bufs=1)
                    for mi in range(4):
                        p1 = pmlp.tile([128, 512], F32, tag="pmlp")
                        nc.tensor.matmul(p1[:], w1t[:, mi * 128:(mi + 1) * 128],
                                         ntA_f, start=True, stop=False)
                        nc.tensor.matmul(p1[:], w1b[:, mi * 128:(mi + 1) * 128],
                                         ntB_f, start=False, stop=True)
                        act.activation(h1t[:, mi, :], p1[:], AF.Relu,
                                       bias=b1t[:, mi:mi + 1])

                    # layer 2
                    h2t = mlp.tile([128, 4, 512], F32R, tag="h2", bufs=1)
                    for mi in range(4):
                        p2 = pmlp.tile([128, 512], F32, tag="pmlp")
                        for k in range(4):
                            nc.tensor.matmul(p2[:], w2t[:, k, mi * 128:(mi + 1) * 128],
                                             h1t[:, k, :], start=(k == 0), stop=(k == 3))
                        act.activation(h2t[:, mi, :], p2[:], AF.Relu,
                                       bias=b2t[:, mi:mi + 1])

                    # layer 3 + extraction into supertile slabs
                    for c in range(C):
                        cc = sub * C + c          # supertile sub-chunk index
                        p3 = pl3.tile([128, 1024], F32, tag="pl3", bufs=2)
                        for k in range(4):
                            lhsT = h2t[:, k, c * 128:(c + 1) * 128]
                            nc.tensor.matmul(p3[:, 0:512], lhsT, w3t[:, k, 0:512],
                                             start=(k == 0), stop=False)
                            nc.tensor.matmul(p3[:, 512:1024], lhsT,
                                             w3t[:, k, 512:1024],
                                             start=(k == 0), stop=False)
                        nc.tensor.matmul(p3[:, 0:512], ones[:], w3l[:, 0:512],
                                         start=False, stop=True)
                        nc.tensor.matmul(p3[:, 512:1024], ones[:], w3l[:, 512:1024],
                                         start=False, stop=True)
                        pb = p3[:].rearrange("p (b x) -> p b x", b=2)
                        act.activation(
                            e4[:, cc, :, :].rearrange("p (b d) j -> p b d j", b=2),
                            pb[:, :, 0:160].rearrange("p b (d j) -> p b d j", j=5),
                            AF.Exp, scale=SCALE)
                        act.activation(
                            e4[:, CS + cc, :, :].rearrange("p (b d) j -> p b d j", b=2),
                            pb[:, :, 160:320].rearrange("p b (d j) -> p b d j", j=5),
                            AF.Exp, scale=SCALE)
                        act.activation(sp[:, cc, 0:32, 0:5],
                                       pb[:, 0, 320:480].rearrange("p (d j) -> p d j", j=5),
                                       AF.Exp)
                        act.activation(sp[:, cc, 32:64, 0:6],
                                       pb[:, 1, 320:512].rearrange("p (d j) -> p d j", j=6),
                                       AF.Exp)

                # ---- merged spline over CS=8 sub-chunks (1024 samples) ----
                x2 = sm.tile([128, CS, 64], F32, tag="x2", bufs=2)
                nc.sync.dma_start(
                    x2[:], d_xn[R0:R0 + 2 * S_TILE, :].rearrange("(c p) f -> p c f",
                                                                 p=128))

                # softplus finish
                act.activation(sp[:, :, 0:32, 0:5], sp[:, :, 0:32, 0:5], AF.Ln,
                               bias=1.0)
                act.activation(sp[:, :, 32:64, 0:6], sp[:, :, 32:64, 0:6], AF.Ln,
                               bias=1.0)
                vec.tensor_scalar_add(sp[:, :, 0:32, 0:5], sp[:, :, 0:32, 0:5], MIN_D)
                vec.tensor_scalar_add(sp[:, :, 32:64, 0:6], sp[:, :, 32:64, 0:6],
                                      MIN_D)
                act.copy(sp[:, :, 0:32, 5], sp[:, :, 0:32, 0])

                # normalized widths/heights, computed in place over e4
                E2 = sm.tile([128, 2 * CS, 64], F32, tag="E2")
                vec.tensor_reduce(E2[:], e4[:], axis=AX.X, op=OP.add)
                rE = sm.tile([128, 2 * CS, 64], F32, tag="rE")
                vec.reciprocal_approx_fast(rE[:], E2[:])
                wh = e4          # overwrite exp values with widths/heights
                vec.scalar_tensor_tensor(
                    wh[:], e4[:], 1.0 - MIN_BW * K,
                    rE[:].unsqueeze(3).broadcast_to([128, 2 * CS, 64, 5]),
                    op0=OP.mult, op1=OP.mult)
                vec.tensor_scalar_add(wh[:], wh[:], MIN_BW)
                w4 = wh[:, 0:CS]
                h4 = wh[:, CS:2 * CS]

                # knots + masks
                Ct = lg.tile([128, CS, 64, 4], F32, tag="Ct", bufs=1)
                vec.tensor_copy(Ct[:, :, :, 0], w4[:, :, :, 0])
                for j in range(1, 4):
                    vec.tensor_add(Ct[:, :, :, j], Ct[:, :, :, j - 1],
                                   w4[:, :, :, j])
                xl = sm.tile([128, CS, 64], F32, tag="xl")
                act.copy(xl[:], x2[:])
                m_t = lg.tile([128, CS, 64, 4], F32, tag="m", bufs=1)
                vec.tensor_tensor(m_t[:],
                                  xl[:].unsqueeze(3).broadcast_to([128, CS, 64, 4]),
                                  Ct[:], op=OP.is_ge)

                dd = lg.tile([128, CS, 64, 5], F32, tag="dd", bufs=1)
                vec.tensor_tensor(dd[:], sp[:, :, :, 1:6], sp[:, :, :, 0:5],
                                  op=OP.subtract)

                # ---- gathers: per quantity, mult + in-place pairwise tree ----
                mf = m_t[:].rearrange("p c d j -> p (c d) j")
                w4f = w4.rearrange("p c d j -> p (c d) j")
                h4f = h4.rearrange("p c d j -> p (c d) j")
                ddf = dd[:].rearrange("p c d j -> p (c d) j")
                VS = [w4f[:, :, 0:4], w4f[:, :, 1:5],
                      h4f[:, :, 0:4], h4f[:, :, 1:5],
                      ddf[:, :, 0:4], ddf[:, :, 1:5]]
                g6 = lg.tile([128, 6, CDS], F32, tag="g6", bufs=1, padded_shape=None)
                for gi, V in enumerate(VS):
                    gtmp = lg.tile([128, CDS, 4], F32, tag="gtmp", name="gtmp",
                                   bufs=1)
                    eng = gps if gi in (2, 3) else vec
                    eng.tensor_tensor(gtmp[:], mf, V, op=OP.mult)
                    vec.tensor_tensor(gtmp[:, :, 0:2], gtmp[:, :, 0:2],
                                      gtmp[:, :, 2:4], op=OP.add)
                    vec.tensor_tensor(g6[:, gi], gtmp[:, :, 0], gtmp[:, :, 1],
                                      op=OP.add)

                def smt(name):
                    return sm.tile([128, CDS], F32, tag=name, name=name)

                w0f = w4[:, :, :, 0].rearrange("p c d -> p (c d)")
                h0f = h4[:, :, :, 0].rearrange("p c d -> p (c d)")
                xlf = x2[:].rearrange("p c d -> p (c d)")

                s = smt("s")
                vec.tensor_tensor(s[:], xlf, g6[:, 0], op=OP.subtract)
                gps.tensor_tensor(g6[:, 1], g6[:, 1], g6[:, 0], op=OP.subtract)
                in_w = smt("in_w")
                vec.tensor_tensor(in_w[:], g6[:, 1], w0f, op=OP.add)
                gps.tensor_tensor(g6[:, 3], g6[:, 3], g6[:, 2], op=OP.subtract)
                in_h = smt("in_h")
                vec.tensor_tensor(in_h[:], g6[:, 3], h0f, op=OP.add)
                ch0 = g6[:, 2]
                dk2 = sm.tile([128, CDS, 2], F32, tag="dk2")
                spf = sp[:].rearrange("p c d j -> p (c d) j")
                g_pair = g6[:].rearrange("p g n -> p n g")[:, :, 4:6]
                vec.tensor_tensor(dk2[:], g_pair, spf[:, :, 0:2], op=OP.add)
                d_k = dk2[:, :, 0]
                d_k1 = dk2[:, :, 1]

                # ---- rational part (heavy in-place tag reuse) ----
                def bin2(name, a, b, op, eng=vec, out=None):
                    tt = out if out is not None else sm.tile([128, CDS], F32,
                                                             tag=name, name=name)
                    eng.tensor_tensor(tt[:], a, b, op=op)
                    return tt

                u = bin2("u", in_w[:], s[:], OP.subtract, gps)
                su = bin2("su", s[:], u[:], OP.mult)
                s2 = s
                act.activation(s2[:], s[:], AF.Square)
                u2 = u
                act.activation(u2[:], u[:], AF.Square)
                w2 = smt("w2")
                act.activation(w2[:], in_w[:], AF.Square)
                t1 = bin2("t1", d_k, d_k1, OP.add, gps)
                t1 = bin2("t1w", t1[:], in_w[:], OP.mult, gps, out=t1)
                hsu = bin2("hsu", in_h[:], su[:], OP.mult)
                hw2 = bin2("hw2", in_h[:], w2[:], OP.mult, gps, out=w2)
                m1 = bin2("m1", t1[:], su[:], OP.mult)
                vec.scalar_tensor_tensor(m1[:], hsu[:], -2.0, m1[:],
                                         op0=OP.mult, op1=OP.add)
                D3 = bin2("D3", m1[:], hw2[:], OP.add)
                t5 = bin2("t5", in_h[:], s2[:], OP.mult)
                t5 = bin2("t5b", in_h[:], t5[:], OP.mult, out=t5)
                aw = bin2("aw", d_k, in_w[:], OP.mult, gps)
                t8 = bin2("t8", hsu[:], aw[:], OP.mult)
                t5 = bin2("num", t5[:], t8[:], OP.add, out=t5)
                rD3 = smt("rD3")
                vec.reciprocal_approx_fast(rD3[:], D3[:])
                t5 = bin2("y0", t5[:], rD3[:], OP.mult, out=t5)
                t5 = bin2("y1", t5[:], ch0, OP.add, out=t5)
                ysc = t5[:].rearrange("p (c d) -> p c d", c=CS)
                vec.tensor_tensor(ysc, ysc,
                                  cW[:].unsqueeze(1).broadcast_to([128, CS, 64]),
                                  op=OP.mult)
                yout = sm.tile([128, CS, 64], F32, tag="yout", bufs=1)
                vec.tensor_tensor(yout[:], ysc,
                                  cL[:].unsqueeze(1).broadcast_to([128, CS, 64]),
                                  op=OP.add)

                bw = bin2("bw", d_k1, in_w[:], OP.mult, gps)
                q1 = bin2("q1", bw[:], s2[:], OP.mult)
                vec.scalar_tensor_tensor(q1[:], hsu[:], 2.0, q1[:],
                                         op0=OP.mult, op1=OP.add)
                q3 = bin2("q3", aw[:], u2[:], OP.mult, gps, out=aw)
                Qt = bin2("Q", q1[:], q3[:], OP.add, out=q1)
                lnh = smt("lnh")
                act.activation(lnh[:], in_h[:], AF.Ln)
                lnw = smt("lnw")
                act.activation(lnw[:], in_w[:], AF.Ln)
                lnQ = smt("lnQ")
                act.activation(lnQ[:], Qt[:], AF.Ln)
                lnD = smt("lnD")
                act.activation(lnD[:], D3[:], AF.Ln)
                vec.scalar_tensor_tensor(lnh[:], lnh[:], 2.0, lnw[:],
                                         op0=OP.mult, op1=OP.add)
                vec.scalar_tensor_tensor(lnD[:], lnD[:], -2.0, lnQ[:],
                                         op0=OP.mult, op1=OP.add)
                ladf = bin2("ladf", lnh[:], lnD[:], OP.add, out=lnh)

                # ---- outputs ----
                dout_t = d_out[R0:R0 + 2 * S_TILE, :].rearrange("(c p) f -> p c f",
                                                                p=128)
                nc.sync.dma_start(dout_t[:, :, 32:64], yout[:, :, 0:32])
                nc.sync.dma_start(dout_t[:, :, 96:128], yout[:, :, 32:64])
                lsum = sm.tile([128, CS], F32, tag="lsum", bufs=2)
                vec.tensor_reduce(lsum[:], ladf[:].rearrange("p (c d) -> p c d", c=CS),
                                  axis=AX.X, op=OP.add)
                nc.sync.dma_start(
                    d_lad[R0:R0 + 2 * S_TILE].rearrange("(c p) -> p c", p=128),
                    lsum[:])

    nc.compile()
    return nc


def prep_weights(W1, b1, W2, b2, W3, b3):
    """Host-side: permute+pad W3 columns, append b3 row, build const tiles."""
    perm = np.zeros(1024, dtype=np.int64)
    valid = np.zeros(1024, dtype=bool)
    for d in range(32):
        for j in range(5):
            perm[0 + d * 5 + j] = d * 15 + j          # uw_c
            perm[160 + d * 5 + j] = d * 15 + 5 + j    # uh_c
            perm[320 + d * 5 + j] = d * 15 + 10 + j   # ud_c
            perm[512 + d * 5 + j] = 480 + d * 16 + j        # uw_r
            perm[672 + d * 5 + j] = 480 + d * 16 + 5 + j    # uh_r
        for j in range(6):
            perm[832 + d * 6 + j] = 480 + d * 16 + 10 + j   # ud_r
    valid[0:480] = True
    valid[512:1024] = True

    W3p = np.zeros((512, 1024), np.float32)
    b3p = np.zeros((1024,), np.float32)
    W3p[:, valid] = W3[:, perm[valid]]
    b3p[valid] = b3[perm[valid]]
    W3b = np.concatenate([W3p, b3p[None, :]], axis=0).astype(np.float32)

    # per-feature constants on the merged 64-wide axis (first 32 circular)
    width = np.where(np.arange(64) < 32, 2.0 * PI, 2.0 * TAIL).astype(np.float32)
    cL = np.broadcast_to(-width / 2.0, (128, 64)).copy()
    cW = np.broadcast_to(width, (128, 64)).copy()

    return {
        "W1f": np.ascontiguousarray(W1, dtype=np.float32),
        "W2f": np.ascontiguousarray(W2, dtype=np.float32),
        "W3b": W3b,
        "b1r": b1.reshape(4, 128).T.copy().astype(np.float32),
        "b2r": b2.reshape(4, 128).T.copy().astype(np.float32),
        "ident": np.eye(128, dtype=np.float32),
        "onesr": np.ones((1, 128), dtype=np.float32),
        "cL": cL, "cW": cW,
    }


def make_xn(inputs):
    xc = (inputs[:, 32:64] + PI) / (2.0 * PI)
    xr = (inputs[:, 96:128] + TAIL) / (2.0 * TAIL)
    return np.ascontiguousarray(np.concatenate([xc, xr], axis=1).astype(np.float32))


def make_net32(inputs, context):
    ic = inputs[:, 0:32]
    return np.ascontiguousarray(np.concatenate(
        [np.cos(ic), np.sin(ic), inputs[:, 64:96], context],
        axis=1).astype(np.float32))


_NC_CACHE = {}


def _get_nc(n_core):
    if n_core not in _NC_CACHE:
        _NC_CACHE[n_core] = build_nc(n_core)
    return _NC_CACHE[n_core]


def kernel(inputs, context, W1, b1, W2, b2, W3, b3):
    inputs = np.ascontiguousarray(np.asarray(inputs, dtype=np.float32))
    context = np.asarray(context, dtype=np.float32)
    wmap = prep_weights(np.asarray(W1, np.float32), np.asarray(b1, np.float32),
                        np.asarray(W2, np.float32), np.asarray(b2, np.float32),
                        np.asarray(W3, np.float32), np.asarray(b3, np.float32))
    net32 = make_net32(inputs, context)
    xn = make_xn(inputs)

    nc = _get_nc(N_CORE)
    in_maps = []
    for c in range(N_CORES):
        sl = slice(c * N_CORE, (c + 1) * N_CORE)
        m = dict(wmap)
        m["inputs"] = inputs[sl]
        m["net32"] = net32[sl]
        m["xn"] = xn[sl]
        in_maps.append(m)

    res = run_bass_kernel_spmd(nc, in_maps, list(range(N_CORES)))
    outputs = np.concatenate([res.results[c]["outputs"] for c in range(N_CORES)], 0)
    lad = np.concatenate([res.results[c]["lad"] for c in range(N_CORES)], 0)
    return outputs, lad


# revision 34
# speedup vs baseline: 1.0148x; 1.0148x over previous
"""Trainium2 Bass kernel for nn_MixedRationalQuadraticCouplingTransform.

kernel(**inputs) takes FULL inputs (N=65536), returns (outputs [N,128] f32,
logabsdet [N] f32). Pure data parallel on 8 NeuronCores: batch sharded 8 ways,
weights replicated.

Per-core program (Tile framework):
  - 16 tiles of 512 samples (4 sub-chunks of 128).
  - net_in (cos/sin/ir/ctx) is precomputed on host; on-chip it is transposed
    to feature-major via PE transposes, then a 3-layer MLP runs with fp32r
    matmul operands (full-rate on the PE, ~1.6e-4 matmul precision).
    Layer 3 uses feature-major h2 as the stationary operand so the 992+pad
    spline params land sample-major in PSUM, with W3 columns host-permuted
    into [uw_c|uh_c|ud_c|pad|uw_r|uh_r|ud_r] blocks and b3 applied via an
    appended ones-row contraction.
  - Both splines (circular + regular) are evaluated by shared ops over a
    64-wide feature axis with per-feature constant tiles; bin search is
    mask-based (m_j = [x >= knot_j]) and gathers are mask-dot products
    evaluated with a pairwise adder tree. All activation functions used
    (Exp/Ln/Relu/Copy/Identity/Square) live in one ACT table so the table is
    loaded exactly once; softplus(x) = Ln(Exp(x) + 1).
"""
import numpy as np

import concourse.bacc as bacc
import concourse.tile as tile
from concourse import mybir
from concourse.bass_utils import run_bass_kernel_spmd

F32 = mybir.dt.float32
F16 = mybir.dt.float16
F32R = mybir.dt.float32r
AX = mybir.AxisListType
OP = mybir.AluOpType
AF = mybir.ActivationFunctionType

N_FULL = 65536
N_CORES = 8
N_CORE = N_FULL // N_CORES          # 8192
S_TILE = 512
C_SUB = S_TILE // 128               # 4
PI = float(np.pi)
SCALE = float(1.0 / np.sqrt(512.0))
MIN_BW = 1e-3
MIN_D = 1e-3
K = 5
TAIL = 5.0
DT_W = F32   # dtype of wide spline tensors

_TABLE_PATCHED = False


def _patch_single_act_table():
    """Force the act-table pass to use only natural_log_exp_and_others
    (covers Exp/Ln/Relu/Copy/Identity/Square) so the table loads once."""
    global _TABLE_PATCHED
    if _TABLE_PATCHED:
        return
    from concourse.hw_specs import get_activation_tables as _orig

    def single(arch):
        tabs = _orig(arch)
        keep = "natural_log_exp_and_others"
        return {k: (v if k == keep else set()) for k, v in tabs.items()}

    bacc.get_activation_tables = single
    _TABLE_PATCHED = True


def build_nc(n_core=N_CORE):
    _patch_single_act_table()
    n_tiles = n_core // S_TILE
    C = C_SUB
    CD = C * 64
    nc = bacc.Bacc("TRN2", target_bir_lowering=False, debug=False,
                   enable_asserts=True, num_devices=1)

    d_in = nc.dram_tensor("inputs", [n_core, 128], F32, kind="ExternalInput").ap()
    d_net = nc.dram_tensor("net32", [n_core, 160], F32R, kind="ExternalInput").ap()
    d_w1 = nc.dram_tensor("W1f", [160, 512], F32R, kind="ExternalInput").ap()
    d_w2 = nc.dram_tensor("W2f", [512, 512], F32R, kind="ExternalInput").ap()
    d_w3 = nc.dram_tensor("W3b", [513, 1024], F32R, kind="ExternalInput").ap()
    d_b1 = nc.dram_tensor("b1r", [128, 4], F32, kind="ExternalInput").ap()
    d_b2 = nc.dram_tensor("b2r", [128, 4], F32, kind="ExternalInput").ap()
    d_id = nc.dram_tensor("ident", [128, 128], F32R, kind="ExternalInput").ap()
    d_ones = nc.dram_tensor("onesr", [1, 128], F32R, kind="ExternalInput").ap()
    # per-feature constants over the merged 64-wide transform axis
    d_cL = nc.dram_tensor("cL", [128, 64], F32, kind="ExternalInput").ap()
    d_cW = nc.dram_tensor("cW", [128, 64], F32, kind="ExternalInput").ap()
    d_xn = nc.dram_tensor("xn", [n_core, 64], F32, kind="ExternalInput").ap()
    d_out = nc.dram_tensor("outputs", [n_core, 128], F32, kind="ExternalOutput").ap()
    d_lad = nc.dram_tensor("lad", [n_core], F32, kind="ExternalOutput").ap()

    with tile.TileContext(nc) as tc:
        with tc.tile_pool(name="const", bufs=1) as cst, \
             tc.tile_pool(name="mlp", bufs=2) as mlp, \
             tc.tile_pool(name="sm", bufs=1) as sm, \
             tc.tile_pool(name="lg", bufs=1) as lg, \
             tc.tile_pool(name="pt", bufs=1, space="PSUM") as ppt, \
             tc.tile_pool(name="pmlp", bufs=3, space="PSUM") as pmlp, \
             tc.tile_pool(name="pl3", bufs=1, space="PSUM") as pl3:

            vec, gps, act = nc.vector, nc.gpsimd, nc.scalar

            # ---- identity feature columns: straight DRAM->DRAM ----
            nc.sync.dma_start(d_out[:, 0:32], d_in[:, 0:32])
            nc.sync.dma_start(d_out[:, 64:96], d_in[:, 64:96])

            # ---- constants ----
            w1t = cst.tile([128, 512], F32R)
            nc.sync.dma_start(w1t[:], d_w1[0:128, :])
            w1b = cst.tile([32, 512], F32R)
            nc.sync.dma_start(w1b[:], d_w1[128:160, :])
            w2t = cst.tile([128, 4, 512], F32R)
            nc.sync.dma_start(w2t[:], d_w2.rearrange("(k p) h -> p k h", p=128))
            w3t = cst.tile([128, 4, 1024], F32R)
            nc.sync.dma_start(w3t[:], d_w3[0:512, :].rearrange("(k p) n -> p k n", p=128))
            w3l = cst.tile([1, 1024], F32R)
            nc.sync.dma_start(w3l[:], d_w3[512:513, :])
            b1t = cst.tile([128, 4], F32)
            nc.sync.dma_start(b1t[:], d_b1[:])
            b2t = cst.tile([128, 4], F32)
            nc.sync.dma_start(b2t[:], d_b2[:])
            idt = cst.tile([128, 128], F32R)
            nc.sync.dma_start(idt[:], d_id[:])
            cL = cst.tile([128, 64], F32)
            nc.sync.dma_start(cL[:], d_cL[:])
            cW = cst.tile([128, 64], F32)
            nc.sync.dma_start(cW[:], d_cW[:])
            ones = cst.tile([1, 128], F32R)
            nc.sync.dma_start(ones[:], d_ones[:])

            for ts_ in range(n_tiles // 2):
                CS = 2 * C          # 8 sub-chunks per spline supertile
                CDS = CS * 64
                R0 = ts_ * 2 * S_TILE
                # spline-wide tiles for the supertile (e4 doubles as wh in place)
                e4 = lg.tile([128, CS * 2, 64, 5], F32, tag="e4", bufs=1)
                sp = lg.tile([128, CS, 64, 6], F32, tag="sp", bufs=1)

                for sub in range(2):
                    r0 = R0 + sub * S_TILE
                    nin = sm.tile([128, C, 160], F32R, tag="nin", bufs=2)
                    nc.sync.dma_start(
                        nin[:],
                        d_net[r0:r0 + S_TILE, :].rearrange("(c p) f -> p c f", p=128))

                    # transpose net_in to feature-major
                    ntA = sm.tile([128, C, 128], F32R, tag="ntA", bufs=2)
                    ntB = sm.tile([32, C, 128], F32R, tag="ntB", bufs=2)
                    for c in range(C):
                        pt = ppt.tile([128, 256], F32R, tag="pt")
                        nc.tensor.transpose(pt[:, 0:128], nin[:, c, 0:128], idt[:])
                        nc.tensor.transpose(pt[0:32, 128:256], nin[:, c, 128:160],
                                            idt[:])
                        act.copy(ntA[:, c, :], pt[:, 0:128])
                        act.copy(ntB[:, c, :], pt[0:32, 128:256])
                    ntA_f = ntA[:].rearrange("p c s -> p (c s)")
                    ntB_f = ntB[:].rearrange("p c s -> p (c s)")

                    # layer 1
                    h1t = mlp.tile([128, 4, 512], F32R, tag="h1", bufs=2)
                    for mi in range(4):
                        p1 = pmlp.tile([128, 512], F32, tag="pmlp")
                        nc.tensor.matmul(p1[:], w1t[:, mi * 128:(mi + 1) * 128],
                                         ntA_f, start=True, stop=False)
                        nc.tensor.matmul(p1[:], w1b[:, mi * 128:(mi + 1) * 128],
                                         ntB_f, start=False, stop=True)
                        act.activation(h1t[:, mi, :], p1[:], AF.Relu,
                                       bias=b1t[:, mi:mi + 1])

                    # layer 2
                    h2t = mlp.tile([128, 4, 512], F32R, tag="h2", bufs=2)
                    for mi in range(4):
                        p2 = pmlp.tile([128, 512], F32, tag="pmlp")
                        for k in range(4):
                            nc.tensor.matmul(p2[:], w2t[:, k, mi * 128:(mi + 1) * 128],
                                             h1t[:, k, :], start=(k == 0), stop=(k == 3))
                        act.activation(h2t[:, mi, :], p2[:], AF.Relu,
                                       bias=b2t[:, mi:mi + 1])

                    # layer 3 + extraction into supertile slabs
                    for c in range(C):
                        cc = sub * C + c          # supertile sub-chunk index
                        p3 = pl3.tile([128, 1024], F32, tag="pl3", bufs=2)
                        for k in range(4):
                            lhsT = h2t[:, k, c * 128:(c + 1) * 128]
                            nc.tensor.matmul(p3[:, 0:512], lhsT, w3t[:, k, 0:512],
                                             start=(k == 0), stop=False)
                            nc.tensor.matmul(p3[:, 512:1024], lhsT,
                                             w3t[:, k, 512:1024],
                                             start=(k == 0), stop=False)
                        nc.tensor.matmul(p3[:, 0:512], ones[:], w3l[:, 0:512],
                                         start=False, stop=True)
                        nc.tensor.matmul(p3[:, 512:1024], ones[:], w3l[:, 512:1024],
                                         start=False, stop=True)
                        pb = p3[:].rearrange("p (b x) -> p b x", b=2)
                        act.activation(
                            e4[:, cc, :, :].rearrange("p (b d) j -> p b d j", b=2),
                            pb[:, :, 0:160].rearrange("p b (d j) -> p b d j", j=5),
                            AF.Exp, scale=SCALE)
                        act.activation(
                            e4[:, CS + cc, :, :].rearrange("p (b d) j -> p b d j", b=2),
                            pb[:, :, 160:320].rearrange("p b (d j) -> p b d j", j=5),
                            AF.Exp, scale=SCALE)
                        act.activation(sp[:, cc, 0:32, 0:5],
                                       pb[:, 0, 320:480].rearrange("p (d j) -> p d j", j=5),
                                       AF.Exp)
                        act.activation(sp[:, cc, 32:64, 0:6],
                                       pb[:, 1, 320:512].rearrange("p (d j) -> p d j", j=6),
                                       AF.Exp)

                # ---- merged spline over CS=8 sub-chunks (1024 samples) ----
                x2 = sm.tile([128, CS, 64], F32, tag="x2", bufs=2)
                nc.sync.dma_start(
                    x2[:], d_xn[R0:R0 + 2 * S_TILE, :].rearrange("(c p) f -> p c f",
                                                                 p=128))

                # softplus finish
                act.activation(sp[:, :, 0:32, 0:5], sp[:, :, 0:32, 0:5], AF.Ln,
                               bias=1.0)
                act.activation(sp[:, :, 32:64, 0:6], sp[:, :, 32:64, 0:6], AF.Ln,
                               bias=1.0)
                vec.tensor_scalar_add(sp[:, :, 0:32, 0:5], sp[:, :, 0:32, 0:5], MIN_D)
                vec.tensor_scalar_add(sp[:, :, 32:64, 0:6], sp[:, :, 32:64, 0:6],
                                      MIN_D)
                act.copy(sp[:, :, 0:32, 5], sp[:, :, 0:32, 0])

                # normalized widths/heights, computed in place over e4
                E2 = sm.tile([128, 2 * CS, 64], F32, tag="E2")
                vec.tensor_reduce(E2[:], e4[:], axis=AX.X, op=OP.add)
                rE = sm.tile([128, 2 * CS, 64], F32, tag="rE")
                vec.reciprocal_approx_fast(rE[:], E2[:])
                wh = e4          # overwrite exp values with widths/heights
                vec.scalar_tensor_tensor(
                    wh[:], e4[:], 1.0 - MIN_BW * K,
                    rE[:].unsqueeze(3).broadcast_to([128, 2 * CS, 64, 5]),
                    op0=OP.mult, op1=OP.mult)
                vec.tensor_scalar_add(wh[:], wh[:], MIN_BW)
                w4 = wh[:, 0:CS]
                h4 = wh[:, CS:2 * CS]

                # knots + masks
                Ct = lg.tile([128, CS, 64, 4], F32, tag="Ct", bufs=1)
                vec.tensor_copy(Ct[:, :, :, 0], w4[:, :, :, 0])
                for j in range(1, 4):
                    vec.tensor_add(Ct[:, :, :, j], Ct[:, :, :, j - 1],
                                   w4[:, :, :, j])
                m_t = lg.tile([128, CS, 64, 4], F32, tag="m", bufs=1)
                vec.tensor_tensor(m_t[:],
                                  x2[:].unsqueeze(3).broadcast_to([128, CS, 64, 4]),
                                  Ct[:], op=OP.is_ge)

                dd = lg.tile([128, CS, 64, 5], F32, tag="Ct", name="dd", bufs=1)
                vec.tensor_tensor(dd[:], sp[:, :, :, 1:6], sp[:, :, :, 0:5],
                                  op=OP.subtract)

                # ---- gathers: per quantity, mult + in-place pairwise tree ----
                mf = m_t[:].rearrange("p c d j -> p (c d) j")
                w4f = w4.rearrange("p c d j -> p (c d) j")
                h4f = h4.rearrange("p c d j -> p (c d) j")
                ddf = dd[:].rearrange("p c d j -> p (c d) j")
                VS = [w4f[:, :, 0:4], w4f[:, :, 1:5],
                      h4f[:, :, 0:4], h4f[:, :, 1:5],
                      ddf[:, :, 0:4], ddf[:, :, 1:5]]
                g6 = lg.tile([128, 6, CDS], F32, tag="g6", bufs=1, padded_shape=None)
                for gi, V in enumerate(VS):
                    gtmp = lg.tile([128, CDS, 4], F32, tag="gtmp", name="gtmp",
                                   bufs=2)
                    eng = gps if gi in (2, 3, 4) else vec
                    eng.tensor_tensor(gtmp[:], mf, V, op=OP.mult)
                    vec.tensor_tensor(gtmp[:, :, 0:2], gtmp[:, :, 0:2],
                                      gtmp[:, :, 2:4], op=OP.add)
                    vec.tensor_tensor(g6[:, gi], gtmp[:, :, 0], gtmp[:, :, 1],
                                      op=OP.add)

                def smt(name):
                    return sm.tile([128, CDS], F32, tag=name, name=name)

                w0f = w4[:, :, :, 0].rearrange("p c d -> p (c d)")
                h0f = h4[:, :, :, 0].rearrange("p c d -> p (c d)")
                xlf = x2[:].rearrange("p c d -> p (c d)")

                s = smt("s")
                vec.tensor_tensor(s[:], xlf, g6[:, 0], op=OP.subtract)
                gps.tensor_tensor(g6[:, 1], g6[:, 1], g6[:, 0], op=OP.subtract)
                in_w = smt("in_w")
                vec.tensor_tensor(in_w[:], g6[:, 1], w0f, op=OP.add)
                gps.tensor_tensor(g6[:, 3], g6[:, 3], g6[:, 2], op=OP.subtract)
                in_h = smt("in_h")
                vec.tensor_tensor(in_h[:], g6[:, 3], h0f, op=OP.add)
                ch0 = g6[:, 2]
                dk2 = sm.tile([128, CDS, 2], F32, tag="dk2")
                spf = sp[:].rearrange("p c d j -> p (c d) j")
                g_pair = g6[:].rearrange("p g n -> p n g")[:, :, 4:6]
                vec.tensor_tensor(dk2[:], g_pair, spf[:, :, 0:2], op=OP.add)
                d_k = dk2[:, :, 0]
                d_k1 = dk2[:, :, 1]

                # ---- rational part (heavy in-place tag reuse) ----
                def bin2(name, a, b, op, eng=vec, out=None):
                    tt = out if out is not None else sm.tile([128, CDS], F32,
                                                             tag=name, name=name)
                    eng.tensor_tensor(tt[:], a, b, op=op)
                    return tt

                u = bin2("u", in_w[:], s[:], OP.subtract, gps)
                su = bin2("su", s[:], u[:], OP.mult)
                s2 = s
                act.activation(s2[:], s[:], AF.Square)
                u2 = u
                act.activation(u2[:], u[:], AF.Square)
                w2 = smt("w2")
                act.activation(w2[:], in_w[:], AF.Square)
                t1 = bin2("t1", d_k, d_k1, OP.add, gps)
                t1 = bin2("t1w", t1[:], in_w[:], OP.mult, gps, out=t1)
                hsu = bin2("hsu", in_h[:], su[:], OP.mult)
                hw2 = bin2("hw2", in_h[:], w2[:], OP.mult, gps, out=w2)
                m1 = bin2("m1", t1[:], su[:], OP.mult)
                vec.scalar_tensor_tensor(m1[:], hsu[:], -2.0, m1[:],
                                         op0=OP.mult, op1=OP.add)
                D3 = bin2("D3", m1[:], hw2[:], OP.add)
                t5 = bin2("t5", in_h[:], s2[:], OP.mult)
                t5 = bin2("t5b", in_h[:], t5[:], OP.mult, out=t5)
                aw = bin2("aw", d_k, in_w[:], OP.mult, gps)
                t8 = bin2("t8", hsu[:], aw[:], OP.mult)
                t5 = bin2("num", t5[:], t8[:], OP.add, out=t5)
                rD3 = smt("rD3")
                vec.reciprocal_approx_fast(rD3[:], D3[:])
                t5 = bin2("y0", t5[:], rD3[:], OP.mult, out=t5)
                t5 = bin2("y1", t5[:], ch0, OP.add, out=t5)
                ysc = t5[:].rearrange("p (c d) -> p c d", c=CS)
                vec.tensor_tensor(ysc, ysc,
                                  cW[:].unsqueeze(1).broadcast_to([128, CS, 64]),
                                  op=OP.mult)
                yout = sm.tile([128, CS, 64], F32, tag="yout", bufs=2)
                vec.tensor_tensor(yout[:], ysc,
                                  cL[:].unsqueeze(1).broadcast_to([128, CS, 64]),
                                  op=OP.add)

                bw = bin2("bw", d_k1, in_w[:], OP.mult, gps, out=t8)
                q1 = bin2("q1", bw[:], s2[:], OP.mult, out=t8)
                vec.scalar_tensor_tensor(q1[:], hsu[:], 2.0, q1[:],
                                         op0=OP.mult, op1=OP.add)
                q3 = bin2("q3", aw[:], u2[:], OP.mult, gps, out=aw)
                Qt = bin2("Q", q1[:], q3[:], OP.add, out=q1)
                act.activation(in_h[:], in_h[:], AF.Ln)
                act.activation(in_w[:], in_w[:], AF.Ln)
                act.activation(Qt[:], Qt[:], AF.Ln)
                act.activation(D3[:], D3[:], AF.Ln)
                vec.scalar_tensor_tensor(in_h[:], in_h[:], 2.0, in_w[:],
                                         op0=OP.mult, op1=OP.add)
                vec.scalar_tensor_tensor(D3[:], D3[:], -2.0, Qt[:],
                                         op0=OP.mult, op1=OP.add)
                ladf = bin2("ladf", in_h[:], D3[:], OP.add, out=in_h)

                # ---- outputs ----
                dout_t = d_out[R0:R0 + 2 * S_TILE, :].rearrange("(c p) f -> p c f",
                                                                p=128)
                nc.sync.dma_start(dout_t[:, :, 32:64], yout[:, :, 0:32])
                nc.sync.dma_start(dout_t[:, :, 96:128], yout[:, :, 32:64])
                lsum = sm.tile([128, CS], F32, tag="lsum", bufs=2)
                vec.tensor_reduce(lsum[:], ladf[:].rearrange("p (c d) -> p c d", c=CS),
                                  axis=AX.X, op=OP.add)
                nc.sync.dma_start(
                    d_lad[R0:R0 + 2 * S_TILE].rearrange("(c p) -> p c", p=128),
                    lsum[:])

    nc.compile()
    return nc


def prep_weights(W1, b1, W2, b2, W3, b3):
    """Host-side: permute+pad W3 columns, append b3 row, build const tiles."""
    perm = np.zeros(1024, dtype=np.int64)
    valid = np.zeros(1024, dtype=bool)
    for d in range(32):
        for j in range(5):
            perm[0 + d * 5 + j] = d * 15 + j          # uw_c
            perm[160 + d * 5 + j] = d * 15 + 5 + j    # uh_c
            perm[320 + d * 5 + j] = d * 15 + 10 + j   # ud_c
            perm[512 + d * 5 + j] = 480 + d * 16 + j        # uw_r
            perm[672 + d * 5 + j] = 480 + d * 16 + 5 + j    # uh_r
        for j in range(6):
            perm[832 + d * 6 + j] = 480 + d * 16 + 10 + j   # ud_r
    valid[0:480] = True
    valid[512:1024] = True

    W3p = np.zeros((512, 1024), np.float32)
    b3p = np.zeros((1024,), np.float32)
    W3p[:, valid] = W3[:, perm[valid]]
    b3p[valid] = b3[perm[valid]]
    W3b = np.concatenate([W3p, b3p[None, :]], axis=0).astype(np.float32)

    # per-feature constants on the merged 64-wide axis (first 32 circular)
    width = np.where(np.arange(64) < 32, 2.0 * PI, 2.0 * TAIL).astype(np.float32)
    cL = np.broadcast_to(-width / 2.0, (128, 64)).copy()
    cW = np.broadcast_to(width, (128, 64)).copy()

    return {
        "W1f": np.ascontiguousarray(W1, dtype=np.float32),
        "W2f": np.ascontiguousarray(W2, dtype=np.float32),
        "W3b": W3b,
        "b1r": b1.reshape(4, 128).T.copy().astype(np.float32),
        "b2r": b2.reshape(4, 128).T.copy().astype(np.float32),
        "ident": np.eye(128, dtype=np.float32),
        "onesr": np.ones((1, 128), dtype=np.float32),
        "cL": cL, "cW": cW,
    }


def make_xn(inputs):
    xc = (inputs[:, 32:64] + PI) / (2.0 * PI)
    xr = (inputs[:, 96:128] + TAIL) / (2.0 * TAIL)
    return np.ascontiguousarray(np.concatenate([xc, xr], axis=1).astype(np.float32))


def make_net32(inputs, context):
    ic = inputs[:, 0:32]
    return np.ascontiguousarray(np.concatenate(
        [np.cos(ic), np.sin(ic), inputs[:, 64:96], context],
        axis=1).astype(np.float32))


_NC_CACHE = {}


def _get_nc(n_core):
    if n_core not in _NC_CACHE:
        _NC_CACHE[n_core] = build_nc(n_core)
    return _NC_CACHE[n_core]


def kernel(inputs, context, W1, b1, W2, b2, W3, b3):
    inputs = np.ascontiguousarray(np.asarray(inputs, dtype=np.float32))
    context = np.asarray(context, dtype=np.float32)
    wmap = prep_weights(np.asarray(W1, np.float32), np.asarray(b1, np.float32),
                        np.asarray(W2, np.float32), np.asarray(b2, np.float32),
                        np.asarray(W3, np.float32), np.asarray(b3, np.float32))
    net32 = make_net32(inputs, context)
    xn = make_xn(inputs)

    nc = _get_nc(N_CORE)
    in_maps = []
    for c in range(N_CORES):
        sl = slice(c * N_CORE, (c + 1) * N_CORE)
        m = dict(wmap)
        m["inputs"] = inputs[sl]
        m["net32"] = net32[sl]
        m["xn"] = xn[sl]
        in_maps.append(m)

    res = run_bass_kernel_spmd(nc, in_maps, list(range(N_CORES)))
    outputs = np.concatenate([res.results[c]["outputs"] for c in range(N_CORES)], 0)
    lad = np.concatenate([res.results[c]["lad"] for c in range(N_CORES)], 0)
    return outputs, lad


# revision 35
# speedup vs baseline: 1.1350x; 1.1184x over previous
"""Trainium2 Bass kernel for nn_MixedRationalQuadraticCouplingTransform.

kernel(**inputs) takes FULL inputs (N=65536), returns (outputs [N,128] f32,
logabsdet [N] f32). Pure data parallel on 8 NeuronCores: batch sharded 8 ways,
weights replicated.

Per-core program (Tile framework):
  - 16 tiles of 512 samples (4 sub-chunks of 128).
  - net_in (cos/sin/ir/ctx) is precomputed on host; on-chip it is transposed
    to feature-major via PE transposes, then a 3-layer MLP runs with fp32r
    matmul operands (full-rate on the PE, ~1.6e-4 matmul precision).
    Layer 3 uses feature-major h2 as the stationary operand so the 992+pad
    spline params land sample-major in PSUM, with W3 columns host-permuted
    into [uw_c|uh_c|ud_c|pad|uw_r|uh_r|ud_r] blocks and b3 applied via an
    appended ones-row contraction.
  - Both splines (circular + regular) are evaluated by shared ops over a
    64-wide feature axis with per-feature constant tiles; bin search is
    mask-based (m_j = [x >= knot_j]) and gathers are mask-dot products
    evaluated with a pairwise adder tree. All activation functions used
    (Exp/Ln/Relu/Copy/Identity/Square) live in one ACT table so the table is
    loaded exactly once; softplus(x) = Ln(Exp(x) + 1).
"""
import numpy as np

import concourse.bacc as bacc
import concourse.tile as tile
from concourse import mybir
from concourse.bass_utils import run_bass_kernel_spmd

F32 = mybir.dt.float32
F16 = mybir.dt.float16
F32R = mybir.dt.float32r
AX = mybir.AxisListType
OP = mybir.AluOpType
AF = mybir.ActivationFunctionType

N_FULL = 65536
N_CORES = 8
N_CORE = N_FULL // N_CORES          # 8192
S_TILE = 512
C_SUB = S_TILE // 128               # 4
PI = float(np.pi)
SCALE = float(1.0 / np.sqrt(512.0))
MIN_BW = 1e-3
MIN_D = 1e-3
K = 5
TAIL = 5.0
DT_W = F32   # dtype of wide spline tensors

_TABLE_PATCHED = False


def _patch_single_act_table():
    """Force the act-table pass to use only natural_log_exp_and_others
    (covers Exp/Ln/Relu/Copy/Identity/Square) so the table loads once."""
    global _TABLE_PATCHED
    if _TABLE_PATCHED:
        return
    from concourse.hw_specs import get_activation_tables as _orig

    def single(arch):
        tabs = _orig(arch)
        keep = "natural_log_exp_and_others"
        return {k: (v if k == keep else set()) for k, v in tabs.items()}

    bacc.get_activation_tables = single
    _TABLE_PATCHED = True


def build_nc(n_core=N_CORE):
    _patch_single_act_table()
    n_tiles = n_core // S_TILE
    C = C_SUB
    CD = C * 64
    nc = bacc.Bacc("TRN2", target_bir_lowering=False, debug=False,
                   enable_asserts=True, num_devices=1)

    d_in = nc.dram_tensor("inputs", [n_core, 128], F32, kind="ExternalInput").ap()
    d_net = nc.dram_tensor("net32", [n_core, 160], F32R, kind="ExternalInput").ap()
    d_w1 = nc.dram_tensor("W1f", [160, 512], F32R, kind="ExternalInput").ap()
    d_w2 = nc.dram_tensor("W2f", [512, 512], F32R, kind="ExternalInput").ap()
    d_w3 = nc.dram_tensor("W3b", [513, 1024], F32R, kind="ExternalInput").ap()
    d_b1 = nc.dram_tensor("b1r", [128, 4], F32, kind="ExternalInput").ap()
    d_b2 = nc.dram_tensor("b2r", [128, 4], F32, kind="ExternalInput").ap()
    d_id = nc.dram_tensor("ident", [128, 128], F32R, kind="ExternalInput").ap()
    d_ones = nc.dram_tensor("onesr", [1, 128], F32R, kind="ExternalInput").ap()
    # per-feature constants over the merged 64-wide transform axis
    d_cL = nc.dram_tensor("cL", [128, 64], F32, kind="ExternalInput").ap()
    d_cW = nc.dram_tensor("cW", [128, 64], F32, kind="ExternalInput").ap()
    d_xn = nc.dram_tensor("xn", [n_core, 64], F32, kind="ExternalInput").ap()
    d_out = nc.dram_tensor("outputs", [n_core, 128], F32, kind="ExternalOutput").ap()
    d_lad = nc.dram_tensor("lad", [n_core], F32, kind="ExternalOutput").ap()

    with tile.TileContext(nc) as tc:
        with tc.tile_pool(name="const", bufs=1) as cst, \
             tc.tile_pool(name="mlp", bufs=2) as mlp, \
             tc.tile_pool(name="sm", bufs=1) as sm, \
             tc.tile_pool(name="lg", bufs=1) as lg, \
             tc.tile_pool(name="pt", bufs=1, space="PSUM") as ppt, \
             tc.tile_pool(name="pmlp", bufs=3, space="PSUM") as pmlp, \
             tc.tile_pool(name="pl3", bufs=1, space="PSUM") as pl3:

            vec, gps, act = nc.vector, nc.gpsimd, nc.scalar

            # ---- identity feature columns: straight DRAM->DRAM ----
            nc.sync.dma_start(d_out[:, 0:32], d_in[:, 0:32])
            nc.sync.dma_start(d_out[:, 64:96], d_in[:, 64:96])

            # ---- constants ----
            w1t = cst.tile([128, 512], F32R)
            nc.sync.dma_start(w1t[:], d_w1[0:128, :])
            w1b = cst.tile([32, 512], F32R)
            nc.sync.dma_start(w1b[:], d_w1[128:160, :])
            w2t = cst.tile([128, 4, 512], F32R)
            nc.sync.dma_start(w2t[:], d_w2.rearrange("(k p) h -> p k h", p=128))
            w3t = cst.tile([128, 4, 1024], F32R)
            nc.sync.dma_start(w3t[:], d_w3[0:512, :].rearrange("(k p) n -> p k n", p=128))
            w3l = cst.tile([1, 1024], F32R)
            nc.sync.dma_start(w3l[:], d_w3[512:513, :])
            b1t = cst.tile([128, 4], F32)
            nc.sync.dma_start(b1t[:], d_b1[:])
            b2t = cst.tile([128, 4], F32)
            nc.sync.dma_start(b2t[:], d_b2[:])
            idt = cst.tile([128, 128], F32R)
            nc.sync.dma_start(idt[:], d_id[:])
            cL = cst.tile([128, 64], F32)
            nc.sync.dma_start(cL[:], d_cL[:])
            cW = cst.tile([128, 64], F32)
            nc.sync.dma_start(cW[:], d_cW[:])
            ones = cst.tile([1, 128], F32R)
            nc.sync.dma_start(ones[:], d_ones[:])

            for ts_ in range(n_tiles // 2):
                CS = 2 * C          # 8 sub-chunks per spline supertile
                CDS = CS * 64
                R0 = ts_ * 2 * S_TILE
                # spline-wide tiles for the supertile (e4 doubles as wh in place)
                e4 = lg.tile([128, CS * 2, 64, 5], F32, tag="e4", bufs=2)
                sp = lg.tile([128, CS, 64, 6], F32, tag="sp", bufs=2)

                for sub in range(2):
                    r0 = R0 + sub * S_TILE
                    nin = sm.tile([128, C, 160], F32R, tag="nin", bufs=1)
                    nc.sync.dma_start(
                        nin[:],
                        d_net[r0:r0 + S_TILE, :].rearrange("(c p) f -> p c f", p=128))

                    # transpose net_in to feature-major
                    ntA = sm.tile([128, C, 128], F32R, tag="ntA", bufs=1)
                    ntB = sm.tile([32, C, 128], F32R, tag="ntB", bufs=1)
                    for c in range(C):
                        pt = ppt.tile([128, 256], F32R, tag="pt")
                        nc.tensor.transpose(pt[:, 0:128], nin[:, c, 0:128], idt[:])
                        nc.tensor.transpose(pt[0:32, 128:256], nin[:, c, 128:160],
                                            idt[:])
                        act.copy(ntA[:, c, :], pt[:, 0:128])
                        act.copy(ntB[:, c, :], pt[0:32, 128:256])
                    ntA_f = ntA[:].rearrange("p c s -> p (c s)")
                    ntB_f = ntB[:].rearrange("p c s -> p (c s)")

                    # layer 1
                    h1t = mlp.tile([128, 4, 512], F32R, tag="h1", bufs=2)
                    for mi in range(4):
                        p1 = pmlp.tile([128, 512], F32, tag="pmlp")
                        nc.tensor.matmul(p1[:], w1t[:, mi * 128:(mi + 1) * 128],
                                         ntA_f, start=True, stop=False)
                        nc.tensor.matmul(p1[:], w1b[:, mi * 128:(mi + 1) * 128],
                                         ntB_f, start=False, stop=True)
                        act.activation(h1t[:, mi, :], p1[:], AF.Relu,
                                       bias=b1t[:, mi:mi + 1])

                    # layer 2
                    h2t = mlp.tile([128, 4, 512], F32R, tag="h2", bufs=1)
                    for mi in range(4):
                        p2 = pmlp.tile([128, 512], F32, tag="pmlp")
                        for k in range(4):
                            nc.tensor.matmul(p2[:], w2t[:, k, mi * 128:(mi + 1) * 128],
                                             h1t[:, k, :], start=(k == 0), stop=(k == 3))
                        act.activation(h2t[:, mi, :], p2[:], AF.Relu,
                                       bias=b2t[:, mi:mi + 1])

                    # layer 3 + extraction into supertile slabs
                    for c in range(C):
                        cc = sub * C + c          # supertile sub-chunk index
                        p3 = pl3.tile([128, 1024], F32, tag="pl3", bufs=2)
                        for k in range(4):
                            lhsT = h2t[:, k, c * 128:(c + 1) * 128]
                            nc.tensor.matmul(p3[:, 0:512], lhsT, w3t[:, k, 0:512],
                                             start=(k == 0), stop=False)
                            nc.tensor.matmul(p3[:, 512:1024], lhsT,
                                             w3t[:, k, 512:1024],
                                             start=(k == 0), stop=False)
                        nc.tensor.matmul(p3[:, 0:512], ones[:], w3l[:, 0:512],
                                         start=False, stop=True)
                        nc.tensor.matmul(p3[:, 512:1024], ones[:], w3l[:, 512:1024],
                                         start=False, stop=True)
                        pb = p3[:].rearrange("p (b x) -> p b x", b=2)
                        act.activation(
                            e4[:, cc, :, :].rearrange("p (b d) j -> p b d j", b=2),
                            pb[:, :, 0:160].rearrange("p b (d j) -> p b d j", j=5),
                            AF.Exp, scale=SCALE)
                        act.activation(
                            e4[:, CS + cc, :, :].rearrange("p (b d) j -> p b d j", b=2),
                            pb[:, :, 160:320].rearrange("p b (d j) -> p b d j", j=5),
                            AF.Exp, scale=SCALE)
                        act.activation(sp[:, cc, 0:32, 0:5],
                                       pb[:, 0, 320:480].rearrange("p (d j) -> p d j", j=5),
                                       AF.Exp)
                        act.activation(sp[:, cc, 32:64, 0:6],
                                       pb[:, 1, 320:512].rearrange("p (d j) -> p d j", j=6),
                                       AF.Exp)

                # ---- merged spline over CS=8 sub-chunks (1024 samples) ----
                x2 = sm.tile([128, CS, 64], F32, tag="x2", bufs=2)
                nc.sync.dma_start(
                    x2[:], d_xn[R0:R0 + 2 * S_TILE, :].rearrange("(c p) f -> p c f",
                                                                 p=128))

                # softplus finish
                act.activation(sp[:, :, 0:32, 0:5], sp[:, :, 0:32, 0:5], AF.Ln,
                               bias=1.0)
                act.activation(sp[:, :, 32:64, 0:6], sp[:, :, 32:64, 0:6], AF.Ln,
                               bias=1.0)
                vec.tensor_scalar_add(sp[:, :, 0:32, 0:5], sp[:, :, 0:32, 0:5], MIN_D)
                vec.tensor_scalar_add(sp[:, :, 32:64, 0:6], sp[:, :, 32:64, 0:6],
                                      MIN_D)
                act.copy(sp[:, :, 0:32, 5], sp[:, :, 0:32, 0])

                # normalized widths/heights, computed in place over e4
                E2 = sm.tile([128, 2 * CS, 64], F32, tag="E2")
                vec.tensor_reduce(E2[:], e4[:], axis=AX.X, op=OP.add)
                rE = sm.tile([128, 2 * CS, 64], F32, tag="rE")
                vec.reciprocal_approx_fast(rE[:], E2[:])
                wh = e4          # overwrite exp values with widths/heights
                vec.scalar_tensor_tensor(
                    wh[:], e4[:], 1.0 - MIN_BW * K,
                    rE[:].unsqueeze(3).broadcast_to([128, 2 * CS, 64, 5]),
                    op0=OP.mult, op1=OP.mult)
                vec.tensor_scalar_add(wh[:], wh[:], MIN_BW)
                w4 = wh[:, 0:CS]
                h4 = wh[:, CS:2 * CS]

                # knots + masks
                Ct = lg.tile([128, CS, 64, 4], F32, tag="Ct", bufs=1)
                vec.tensor_copy(Ct[:, :, :, 0], w4[:, :, :, 0])
                for j in range(1, 4):
                    vec.tensor_add(Ct[:, :, :, j], Ct[:, :, :, j - 1],
                                   w4[:, :, :, j])
                m_t = lg.tile([128, CS, 64, 4], F16, tag="m", bufs=1)
                vec.tensor_tensor(m_t[:],
                                  x2[:].unsqueeze(3).broadcast_to([128, CS, 64, 4]),
                                  Ct[:], op=OP.is_ge)

                dd = lg.tile([128, CS, 64, 5], F32, tag="Ct", name="dd", bufs=1)
                vec.tensor_tensor(dd[:], sp[:, :, :, 1:6], sp[:, :, :, 0:5],
                                  op=OP.subtract)

                # ---- gathers: per quantity, mult + in-place pairwise tree ----
                mf = m_t[:].rearrange("p c d j -> p (c d) j")
                w4f = w4.rearrange("p c d j -> p (c d) j")
                h4f = h4.rearrange("p c d j -> p (c d) j")
                ddf = dd[:].rearrange("p c d j -> p (c d) j")
                VS = [w4f[:, :, 0:4], w4f[:, :, 1:5],
                      h4f[:, :, 0:4], h4f[:, :, 1:5],
                      ddf[:, :, 0:4], ddf[:, :, 1:5]]
                g6 = lg.tile([128, 6, CDS], F32, tag="g6", bufs=1, padded_shape=None)
                for gi, V in enumerate(VS):
                    gtmp = lg.tile([128, CDS, 4], F32, tag="gtmp", name="gtmp",
                                   bufs=1)
                    eng = gps if gi in (2, 3, 4) else vec
                    eng.tensor_tensor(gtmp[:], mf, V, op=OP.mult)
                    vec.tensor_tensor(gtmp[:, :, 0:2], gtmp[:, :, 0:2],
                                      gtmp[:, :, 2:4], op=OP.add)
                    vec.tensor_tensor(g6[:, gi], gtmp[:, :, 0], gtmp[:, :, 1],
                                      op=OP.add)

                def smt(name):
                    return sm.tile([128, CDS], F32, tag=name, name=name)

                w0f = w4[:, :, :, 0].rearrange("p c d -> p (c d)")
                h0f = h4[:, :, :, 0].rearrange("p c d -> p (c d)")
                xlf = x2[:].rearrange("p c d -> p (c d)")

                s = smt("s")
                vec.tensor_tensor(s[:], xlf, g6[:, 0], op=OP.subtract)
                gps.tensor_tensor(g6[:, 1], g6[:, 1], g6[:, 0], op=OP.subtract)
                in_w = smt("in_w")
                vec.tensor_tensor(in_w[:], g6[:, 1], w0f, op=OP.add)
                gps.tensor_tensor(g6[:, 3], g6[:, 3], g6[:, 2], op=OP.subtract)
                in_h = smt("in_h")
                vec.tensor_tensor(in_h[:], g6[:, 3], h0f, op=OP.add)
                ch0 = g6[:, 2]
                dk2 = sm.tile([128, CDS, 2], F32, tag="dk2")
                spf = sp[:].rearrange("p c d j -> p (c d) j")
                g_pair = g6[:].rearrange("p g n -> p n g")[:, :, 4:6]
                vec.tensor_tensor(dk2[:], g_pair, spf[:, :, 0:2], op=OP.add)
                d_k = dk2[:, :, 0]
                d_k1 = dk2[:, :, 1]

                # ---- rational part (heavy in-place tag reuse) ----
                def bin2(name, a, b, op, eng=vec, out=None):
                    tt = out if out is not None else sm.tile([128, CDS], F32,
                                                             tag=name, name=name)
                    eng.tensor_tensor(tt[:], a, b, op=op)
                    return tt

                u = bin2("u", in_w[:], s[:], OP.subtract, gps)
                su = bin2("su", s[:], u[:], OP.mult)
                s2 = s
                act.activation(s2[:], s[:], AF.Square)
                u2 = u
                act.activation(u2[:], u[:], AF.Square)
                w2 = smt("w2")
                act.activation(w2[:], in_w[:], AF.Square)
                t1 = bin2("t1", d_k, d_k1, OP.add, gps)
                t1 = bin2("t1w", t1[:], in_w[:], OP.mult, gps, out=t1)
                hsu = bin2("hsu", in_h[:], su[:], OP.mult)
                hw2 = bin2("hw2", in_h[:], w2[:], OP.mult, gps, out=w2)
                m1 = bin2("m1", t1[:], su[:], OP.mult)
                vec.scalar_tensor_tensor(m1[:], hsu[:], -2.0, m1[:],
                                         op0=OP.mult, op1=OP.add)
                D3 = bin2("D3", m1[:], hw2[:], OP.add)
                t5 = bin2("t5", in_h[:], s2[:], OP.mult)
                t5 = bin2("t5b", in_h[:], t5[:], OP.mult, out=t5)
                aw = bin2("aw", d_k, in_w[:], OP.mult, gps, out=t1)
                t8 = bin2("t8", hsu[:], aw[:], OP.mult)
                t5 = bin2("num", t5[:], t8[:], OP.add, out=t5)
                rD3 = m1
                vec.reciprocal_approx_fast(rD3[:], D3[:])
                t5 = bin2("y0", t5[:], rD3[:], OP.mult, out=t5)
                t5 = bin2("y1", t5[:], ch0, OP.add, out=t5)
                ysc = t5[:].rearrange("p (c d) -> p c d", c=CS)
                vec.tensor_tensor(ysc, ysc,
                                  cW[:].unsqueeze(1).broadcast_to([128, CS, 64]),
                                  op=OP.mult)
                yout = sm.tile([128, CS, 64], F32, tag="yout", bufs=1)
                vec.tensor_tensor(yout[:], ysc,
                                  cL[:].unsqueeze(1).broadcast_to([128, CS, 64]),
                                  op=OP.add)

                bw = bin2("bw", d_k1, in_w[:], OP.mult, gps, out=t8)
                q1 = bin2("q1", bw[:], s2[:], OP.mult, out=t8)
                vec.scalar_tensor_tensor(q1[:], hsu[:], 2.0, q1[:],
                                         op0=OP.mult, op1=OP.add)
                q3 = bin2("q3", aw[:], u2[:], OP.mult, gps, out=aw)
                Qt = bin2("Q", q1[:], q3[:], OP.add, out=q1)
                act.activation(in_h[:], in_h[:], AF.Ln)
                act.activation(in_w[:], in_w[:], AF.Ln)
                act.activation(Qt[:], Qt[:], AF.Ln)
                act.activation(D3[:], D3[:], AF.Ln)
                vec.scalar_tensor_tensor(in_h[:], in_h[:], 2.0, in_w[:],
                                         op0=OP.mult, op1=OP.add)
                vec.scalar_tensor_tensor(D3[:], D3[:], -2.0, Qt[:],
                                         op0=OP.mult, op1=OP.add)
                ladf = bin2("ladf", in_h[:], D3[:], OP.add, out=in_h)

                # ---- outputs ----
                dout_t = d_out[R0:R0 + 2 * S_TILE, :].rearrange("(c p) f -> p c f",
                                                                p=128)
                nc.sync.dma_start(dout_t[:, :, 32:64], yout[:, :, 0:32])
                nc.sync.dma_start(dout_t[:, :, 96:128], yout[:, :, 32:64])
                lsum = sm.tile([128, CS], F32, tag="lsum", bufs=1)
                vec.tensor_reduce(lsum[:], ladf[:].rearrange("p (c d) -> p c d", c=CS),
                                  axis=AX.X, op=OP.add)
                nc.sync.dma_start(
                    d_lad[R0:R0 + 2 * S_TILE].rearrange("(c p) -> p c", p=128),
                    lsum[:])

    nc.compile()
    return nc


def prep_weights(W1, b1, W2, b2, W3, b3):
    """Host-side: permute+pad W3 columns, append b3 row, build const tiles."""
    perm = np.zeros(1024, dtype=np.int64)
    valid = np.zeros(1024, dtype=bool)
    for d in range(32):
        for j in range(5):
            perm[0 + d * 5 + j] = d * 15 + j          # uw_c
            perm[160 + d * 5 + j] = d * 15 + 5 + j    # uh_c
            perm[320 + d * 5 + j] = d * 15 + 10 + j   # ud_c
            perm[512 + d * 5 + j] = 480 + d * 16 + j        # uw_r
            perm[672 + d * 5 + j] = 480 + d * 16 + 5 + j    # uh_r
        for j in range(6):
            perm[832 + d * 6 + j] = 480 + d * 16 + 10 + j   # ud_r
    valid[0:480] = True
    valid[512:1024] = True

    W3p = np.zeros((512, 1024), np.float32)
    b3p = np.zeros((1024,), np.float32)
    W3p[:, valid] = W3[:, perm[valid]]
    b3p[valid] = b3[perm[valid]]
    W3b = np.concatenate([W3p, b3p[None, :]], axis=0).astype(np.float32)

    # per-feature constants on the merged 64-wide axis (first 32 circular)
    width = np.where(np.arange(64) < 32, 2.0 * PI, 2.0 * TAIL).astype(np.float32)
    cL = np.broadcast_to(-width / 2.0, (128, 64)).copy()
    cW = np.broadcast_to(width, (128, 64)).copy()

    return {
        "W1f": np.ascontiguousarray(W1, dtype=np.float32),
        "W2f": np.ascontiguousarray(W2, dtype=np.float32),
        "W3b": W3b,
        "b1r": b1.reshape(4, 128).T.copy().astype(np.float32),
        "b2r": b2.reshape(4, 128).T.copy().astype(np.float32),
        "ident": np.eye(128, dtype=np.float32),
        "onesr": np.ones((1, 128), dtype=np.float32),
        "cL": cL, "cW": cW,
    }


def make_xn(inputs):
    xc = (inputs[:, 32:64] + PI) / (2.0 * PI)
    xr = (inputs[:, 96:128] + TAIL) / (2.0 * TAIL)
    return np.ascontiguousarray(np.concatenate([xc, xr], axis=1).astype(np.float32))


def make_net32(inputs, context):
    ic = inputs[:, 0:32]
    return np.ascontiguousarray(np.concatenate(
        [np.cos(ic), np.sin(ic), inputs[:, 64:96], context],
        axis=1).astype(np.float32))


_NC_CACHE = {}


def _get_nc(n_core):
    if n_core not in _NC_CACHE:
        _NC_CACHE[n_core] = build_nc(n_core)
    return _NC_CACHE[n_core]


def kernel(inputs, context, W1, b1, W2, b2, W3, b3):
    inputs = np.ascontiguousarray(np.asarray(inputs, dtype=np.float32))
    context = np.asarray(context, dtype=np.float32)
    wmap = prep_weights(np.asarray(W1, np.float32), np.asarray(b1, np.float32),
                        np.asarray(W2, np.float32), np.asarray(b2, np.float32),
                        np.asarray(W3, np.float32), np.asarray(b3, np.float32))
    net32 = make_net32(inputs, context)
    xn = make_xn(inputs)

    nc = _get_nc(N_CORE)
    in_maps = []
    for c in range(N_CORES):
        sl = slice(c * N_CORE, (c + 1) * N_CORE)
        m = dict(wmap)
        m["inputs"] = inputs[sl]
        m["net32"] = net32[sl]
        m["xn"] = xn[sl]
        in_maps.append(m)

    res = run_bass_kernel_spmd(nc, in_maps, list(range(N_CORES)))
    outputs = np.concatenate([res.results[c]["outputs"] for c in range(N_CORES)], 0)
    lad = np.concatenate([res.results[c]["lad"] for c in range(N_CORES)], 0)
    return outputs, lad
